# revision 83
# baseline (speedup 1.0000x reference)
"""Channel-attention (nn_ChannelAttentionModule) Trainium2 kernel.

Math (per batch b, C=512, N=64*64=4096):
    X = x[b]  [C, N]
    q = Wq X + bq ; k = Wk X + bk ; v = Wv X + bv
    L = q k^T ; A = softmax(L, -1) ; out = A v + X

Restructure:
    L  = Wq G Wk^T + bq (Wk S + N bk)^T + (Wq S) bk^T
         with G = X X^T (fp16 Gram, symmetric; upper block-triangle on the
         PE, lower blocks mirrored via PE transposes) and S = X 1 (spatial
         row sums, accumulated as free ap=1 PE matmuls against ones).
         T1 = G Wk^T, L = Wq T1, and the rank-1 bias terms all run as
         single fp32r passes (1 cyc/row at ap>=256, full fp32 inputs).
    A v = rcp (.) [ (E Wv) X + (E bv) 1^T ]
         with E = exp(L - rowmax) fp16, rcp = 1/rowsum(E).  Reassociating
         (E Wv) X kills the C^2 N v-conv entirely: M0 = E Wv is only C^3,
         computed strip-wise from PE-transposed E^T; E bv is a DVE row-dot.
    out = Identity(o_ps * rcp + rcp*E bv) + X   (scalar-engine scale/bias,
         vector-engine residual add, fp16 store; host casts to fp32).

Sharding: pure data-parallel, one batch per NeuronCore (B=8, 8 cores).
"""

import numpy as np

import concourse.mybir as mybir
import concourse.tile as tile
from concourse import bacc
from concourse.bass_utils import run_bass_kernel_spmd

F32 = mybir.dt.float32
F32R = mybir.dt.float32r
F16 = mybir.dt.float16
AX = mybir.AxisListType.X
EXP = mybir.ActivationFunctionType.Exp
IDENT = mybir.ActivationFunctionType.Identity

B = 8
C = 512
HW = 64 * 64
P = 128
CH = C // P  # 4 channel chunks
NG = 8  # xt granules (4 spatial tiles of 128 rows each)
# upper-triangle start per G row chunk
USTART = [0, 128, 256, 384]
# lower blocks (c,d) mirrored from upper gh[d][:, c-block]
MIRROR = [(1, 0), (2, 0), (2, 1), (3, 0), (3, 1), (3, 2)]
OTAGS = ["pa0", "pa1", "pa2", "pa3", "pb0", "pb1", "pb2", "pb3"]


def _body(tc, nc, io):
    xt16, x16 = io["xt16"], io["x16"]
    wqt, wkt, wv = io["wqt"], io["wkt"], io["wv"]
    bqr, bkr, nbkr, bv16 = io["bqr"], io["bkr"], io["nbkr"], io["bv16"]
    id16, id32, ones16, out = io["id16"], io["id32"], io["ones16"], io["out"]

    ps = tc.alloc_tile_pool(name="ps", bufs=1, space="PSUM")
    sb = tc.alloc_tile_pool(name="sb", bufs=1)

    # ---- persistent SBUF tiles ----
    x16_sb = [sb.tile([P, HW], F16, name=f"x16_{i}", tag=f"x16_{i}") for i in range(CH)]
    wqt_sb = sb.tile([P, CH * C], F32R, name="wqt_sb", tag="wqt_sb")
    wkt_sb = sb.tile([P, CH * C], F32R, name="wkt_sb", tag="wkt_sb")
    wv_sb = sb.tile([P, CH * C], F16, name="wv_sb", tag="wv_sb")
    bvb_sb = sb.tile([P, C], F16, name="bvb_sb", tag="bvb_sb")
    id16_sb = sb.tile([P, P], F16, name="id16_sb", tag="id16_sb")
    id32_sb = sb.tile([P, P], F32R, name="id32_sb", tag="id32_sb")
    ones_sb = sb.tile([P, 1], F16, name="ones_sb", tag="ones_sb")
    CA = C + 1  # x columns + a ones column (S rides along in the G matmuls)
    ar_sb = [sb.tile([P, 4 * CA], F16, name=f"ar{g}", tag=f"ar{g}") for g in range(NG)]
    gf = [sb.tile([P, C], F32R, name=f"gf{i}", tag=f"gf{i}") for i in range(CH)]
    t1f = [sb.tile([P, C], F32R, name=f"t1f{i}", tag=f"t1f{i}") for i in range(CH)]
    e16 = [sb.tile([P, C], F16, name=f"e16_{i}", tag=f"e16_{i}") for i in range(CH)]
    et_sb = [sb.tile([P, C], F16, name=f"et{j}", tag=f"et{j}") for j in range(CH)]
    m0t = [sb.tile([P, C], F16, name=f"m0t{j}", tag=f"m0t{j}") for j in range(CH)]
    s32 = [sb.tile([P, 1], F32R, name=f"s32_{i}", tag=f"s32_{i}") for i in range(CH)]
    bq_sb = sb.tile([1, C], F32R, name="bq_sb", tag="bq_sb")
    bk_sb = sb.tile([1, C], F32R, name="bk_sb", tag="bk_sb")
    u2n_sb = sb.tile([1, C], F32R, name="u2n_sb", tag="u2n_sb")
    nbkr_sb = sb.tile([1, C], F32, name="nbkr_sb", tag="nbkr_sb")
    u1f_sb = sb.tile([1, C], F32R, name="u1f_sb", tag="u1f_sb")

    def wslice(tile_, e, lo, hi):
        return tile_[:, e * C + lo : e * C + hi]

    # ---- DMA schedule (SP queue order == DMA device order) ----
    xtr3 = xt16.rearrange("(g t p) c -> g p t c", p=P, t=4)
    for q in range(4):  # quarter granule 0 so G starts ASAP
        nc.sync.dma_start(ar_sb[0][:, q * CA : (q + 1) * CA], xtr3[0][:, q])
    nc.sync.dma_start(
        ar_sb[1][:, 0 : 2 * CA].rearrange("p (t c) -> p t c", t=2), xtr3[1][:, 0:2]
    )
    nc.sync.dma_start(
        ar_sb[1][:, 2 * CA :].rearrange("p (t c) -> p t c", t=2), xtr3[1][:, 2:4]
    )
    nc.sync.dma_start(
        ar_sb[2][:, 0 : 2 * CA].rearrange("p (t c) -> p t c", t=2), xtr3[2][:, 0:2]
    )
    nc.sync.dma_start(
        ar_sb[2][:, 2 * CA :].rearrange("p (t c) -> p t c", t=2), xtr3[2][:, 2:4]
    )
    nc.sync.dma_start(
        ar_sb[3][:, 0 : 2 * CA].rearrange("p (t c) -> p t c", t=2), xtr3[3][:, 0:2]
    )
    nc.sync.dma_start(
        ar_sb[3][:, 2 * CA :].rearrange("p (t c) -> p t c", t=2), xtr3[3][:, 2:4]
    )
    nc.sync.dma_start(
        ar_sb[4][:, 0 : 2 * CA].rearrange("p (t c) -> p t c", t=2), xtr3[4][:, 0:2]
    )
    nc.sync.dma_start(
        ar_sb[4][:, 2 * CA :].rearrange("p (t c) -> p t c", t=2), xtr3[4][:, 2:4]
    )
    for g in range(5, NG):
        nc.sync.dma_start(ar_sb[g].rearrange("p (t c) -> p t c", t=4), xtr3[g])
    nc.sync.dma_start(id16_sb, id16)
    nc.sync.dma_start(id32_sb, id32)
    nc.sync.dma_start(ones_sb, ones16)
    nc.sync.dma_start(
        wkt_sb.rearrange("p (e c) -> p e c", e=CH),
        wkt.rearrange("(e p) c -> p e c", p=P),
    )
    nc.sync.dma_start(
        wqt_sb.rearrange("p (e c) -> p e c", e=CH),
        wqt.rearrange("(e p) c -> p e c", p=P),
    )
    nc.sync.dma_start(bq_sb, bqr)
    nc.sync.dma_start(bk_sb, bkr)
    nc.sync.dma_start(nbkr_sb, nbkr)
    nc.sync.dma_start(
        wv_sb.rearrange("p (e c) -> p e c", e=CH),
        wv.rearrange("(e p) c -> p e c", p=P),
    )
    nc.sync.dma_start(bvb_sb, bv16)
    for c in range(CH):
        nc.sync.dma_start(x16_sb[c], x16[c * P : (c + 1) * P, :])

    # ---- G = X X^T (upper block-triangle) + S = X 1, accumulated on PE.
    #      For c>=1 the rhs includes the trailing ones column, so S[c-block]
    #      lands in the psum tile's last column for free; c=0 would need a
    #      513-wide bank, so it keeps a separate ap=1 accumulation. ----
    g_ps = [
        ps.tile([P, C + 1 - USTART[i] if i else C], F32, name=f"gps{i}", tag=f"pa{i}")
        for i in range(CH)
    ]
    s_ps0 = ps.tile([P, 1], F32, name="s_ps0", tag="pb0")

    for g2 in range(NG):
        ar4 = ar_sb[g2]
        for t in range(4):
            n = g2 * 4 + t
            first, last = n == 0, n == 4 * NG - 1
            for c in range(CH):
                u = USTART[c]
                nc.tensor.matmul(
                    g_ps[c],
                    lhsT=ar4[:, t * CA + c * P : t * CA + (c + 1) * P],
                    rhs=ar4[:, t * CA + u : t * CA + (C + 1 if c else C)],
                    start=first,
                    stop=last,
                )
            nc.tensor.matmul(
                s_ps0,
                lhsT=ar4[:, t * CA : t * CA + P],
                rhs=ones_sb,
                start=first,
                stop=last,
            )

    # ---- gf = fp32 G in SBUF; mirror lower blocks via fp32 PE transposes ----
    nc.vector.tensor_copy(s32[0], s_ps0)
    for i in range(1, CH):
        nc.vector.tensor_copy(s32[i], g_ps[i][:, C - USTART[i] : C + 1 - USTART[i]])
    # PSUM->SBUF copies split across Act/DVE; the sub-blocks the mirror
    # transposes read are copied first so the mirrors can start early.
    # (g_ps[c] for c>=1 is USTART-offset and 1 col wider for the S column.)
    def gcopy(c, lo, hi):
        u = USTART[c]
        dst = gf[c][:, lo:hi]
        srcp = g_ps[c][:, lo - u : hi - u] if c else g_ps[c][:, lo:hi]
        if c % 2 == 0:
            nc.scalar.copy(dst, srcp)
        else:
            nc.vector.tensor_copy(dst, srcp)

    gcopy(0, 128, 256)   # feeds mirror (1,0)
    gcopy(1, 256, 384)   # feeds mirror (2,1)
    gcopy(0, 256, 512)   # feeds mirrors (2,0), (3,0)
    gcopy(1, 384, 512)   # feeds mirror (3,1)
    gcopy(2, 384, 512)   # feeds mirror (3,2)
    gcopy(0, 0, 128)
    gcopy(1, 128, 256)
    gcopy(2, 256, 384)
    gcopy(3, 384, 512)
    for idx, (c, d) in enumerate(MIRROR):
        tb = ps.tile(
            [P, P], F32R, name="tb", tag="pb3" if idx % 2 == 0 else "pb0"
        )
        nc.tensor.transpose(tb, gf[d][:, c * P : (c + 1) * P], id32_sb)
        nc.scalar.copy(gf[c][:, d * P : (d + 1) * P], tb)

    # ---- u1 = (Wq S)^T, u2 = (Wk S)^T (fp32r); rank-2 bias factors ----
    u1_ps = ps.tile([1, C], F32, name="u1_ps", tag="pb1")
    u2_ps = ps.tile([1, C], F32, name="u2_ps", tag="pb2")
    for e in range(CH):
        nc.tensor.matmul(
            u1_ps, lhsT=s32[e], rhs=wslice(wqt_sb, e, 0, C),
            start=e == 0, stop=e == CH - 1,
        )
    for e in range(CH):
        nc.tensor.matmul(
            u2_ps, lhsT=s32[e], rhs=wslice(wkt_sb, e, 0, C),
            start=e == 0, stop=e == CH - 1,
        )
    nc.scalar.copy(u1f_sb, u1_ps)
    nc.vector.tensor_add(u2n_sb, u2_ps, nbkr_sb)

    # ---- T1 = G Wk^T (single fp32r pass; lhsT = G blocks via symmetry),
    #      interleaved with the logits groups so exp starts ASAP ----
    t1_ps = [ps.tile([P, C], F32, name=f"t1ps{e}", tag=f"pa{e}") for e in range(CH)]

    # ---- logits = Wq T1 + rank-2 (all fp32r); softmax row stats ----
    l_ps = [ps.tile([P, C], F32, name=f"lps{c}", tag=f"pb{c}") for c in range(CH)]
    negmx = [sb.tile([P, 1], F32, name=f"negmx{c}", tag=f"negmx{c}") for c in range(CH)]
    ssum = [sb.tile([P, 1], F32, name=f"ssum{c}", tag=f"ssum{c}") for c in range(CH)]
    rcp = [sb.tile([P, 1], F32, name=f"rcp{c}", tag=f"rcp{c}") for c in range(CH)]
    r0f = [sb.tile([P, 1], F32, name=f"r0f{c}", tag=f"r0f{c}") for c in range(CH)]
    rr = [sb.tile([P, 1], F32, name=f"rr{c}", tag=f"rr{c}") for c in range(CH)]
    for e in range(CH):
        for f in range(CH):
            nc.tensor.matmul(
                t1_ps[e], lhsT=gf[f][:, e * P : (e + 1) * P],
                rhs=wslice(wkt_sb, f, 0, C), start=f == 0, stop=f == CH - 1,
            )
        if e % 2 == 1:
            nc.scalar.copy(t1f[e], t1_ps[e])
        else:
            nc.vector.tensor_copy(t1f[e], t1_ps[e])
    for c in range(CH):
        for e in range(CH):
            nc.tensor.matmul(
                l_ps[c], lhsT=wslice(wqt_sb, e, c * P, (c + 1) * P),
                rhs=t1f[e], start=e == 0, stop=False,
            )
        # rank-1 bias terms last: bq (x) (u2 + N bk)  and  u1 (x) bk
        nc.tensor.matmul(
            l_ps[c], lhsT=bq_sb[:, c * P : (c + 1) * P], rhs=u2n_sb,
            start=False, stop=False,
        )
        nc.tensor.matmul(
            l_ps[c], lhsT=u1f_sb[:, c * P : (c + 1) * P], rhs=bk_sb,
            start=False, stop=True,
        )
        nc.vector.reduce_max(negmx[c], l_ps[c], axis=AX, negate=True)
        nc.scalar.activation(
            e16[c], l_ps[c], EXP, bias=negmx[c], scale=1.0, accum_out=ssum[c]
        )
        nc.vector.reciprocal(rcp[c], ssum[c])

    # ---- E^T via fp16 PE transposes + M0^T = Wv^T E^T, strip-pipelined ----
    et_ps = [ps.tile([P, C], F16, name=f"etps{j}", tag=f"pb{j}") for j in range(CH)]
    m0t_ps = [ps.tile([P, C], F32, name=f"m0tps{d}", tag=f"pa{d}") for d in range(CH)]
    def strip_transp(c):
        cs = slice(c * P, (c + 1) * P)
        for j in range(CH):
            nc.tensor.transpose(
                et_ps[j][:, cs], e16[c][:, j * P : (j + 1) * P], id16_sb
            )
        for j in range(CH):
            if j % 2 == 0:
                nc.scalar.copy(et_sb[j][:, cs], et_ps[j][:, cs])
            else:
                nc.vector.tensor_copy(et_sb[j][:, cs], et_ps[j][:, cs])

    def strip_m0t(c):
        cs = slice(c * P, (c + 1) * P)
        for d in range(CH):
            for e in range(CH):
                nc.tensor.matmul(
                    m0t_ps[d][:, cs], lhsT=wslice(wv_sb, e, d * P, (d + 1) * P),
                    rhs=et_sb[e][:, cs], start=e == 0, stop=e == CH - 1,
                )
        for d in range(CH):
            if d % 2 == 0:
                nc.scalar.copy(m0t[d][:, cs], m0t_ps[d][:, cs])
            else:
                nc.vector.tensor_copy(m0t[d][:, cs], m0t_ps[d][:, cs])

    strip_transp(0)
    strip_transp(1)
    strip_m0t(0)
    strip_transp(2)
    strip_m0t(1)
    strip_transp(3)
    strip_m0t(2)
    strip_m0t(3)

    # ---- rr = rcp * (E bv) via DVE row-dot ----
    for c in range(CH):
        ttrs = sb.tile([P, C], F16, name="ttrs", tag="ttrs", bufs=2)
        nc.vector.tensor_tensor(ttrs, e16[c], bvb_sb, mybir.AluOpType.mult)
        nc.vector.reduce_sum(r0f[c], ttrs, axis=AX)
        nc.vector.tensor_scalar_mul(rr[c], r0f[c], rcp[c])

    # ---- out = rcp*(M0^T^T X) + rr + X, fp16 store per 2-tile chunk ----
    for c in range(CH):
        o16 = sb.tile([P, HW], F16, name="o16", tag="o16", bufs=2)
        for nt in range(HW // 512):
            o_ps = ps.tile(
                [P, 512], F32, name="o_ps", tag=OTAGS[(c * 8 + nt) % 8]
            )
            for d in range(CH):
                nc.tensor.matmul(
                    o_ps,
                    lhsT=m0t[d][:, c * P : (c + 1) * P],
                    rhs=x16_sb[d][:, nt * 512 : (nt + 1) * 512],
                    start=d == 0,
                    stop=d == CH - 1,
                )
            t_sb = sb.tile([P, 512], F16, name="t_sb", tag="t_sb", bufs=4)
            nc.scalar.activation(t_sb, o_ps, IDENT, bias=rr[c], scale=rcp[c])
            nc.vector.tensor_add(
                o16[:, nt * 512 : (nt + 1) * 512],
                t_sb,
                x16_sb[c][:, nt * 512 : (nt + 1) * 512],
            )
            if c == CH - 1:  # tail: store per tile so the last chunk is small
                q = nt * 512
                eng = nc.gpsimd if nt % 2 == 0 else nc.sync
                eng.dma_start(
                    out[c * P : (c + 1) * P, q : q + 512], o16[:, q : q + 512]
                )
            elif nt % 2 == 1:
                q = (nt - 1) * 512
                eng = nc.gpsimd if (c * 4 + nt // 2) % 2 == 0 else nc.sync
                eng.dma_start(
                    out[c * P : (c + 1) * P, q : q + 1024], o16[:, q : q + 1024]
                )

    for pool in (sb, ps):
        pool.release()


def _build_nc(repeat=1):
    nc = bacc.Bacc(
        "TRN2",
        target_bir_lowering=False,
        debug=False,
        num_devices=B,
        enable_asserts=False,
    )
    io = {}
    dt = nc.dram_tensor
    io["xt16"] = dt("xt16", (HW, C + 1), F16, kind="ExternalInput").ap()
    io["x16"] = dt("x16", (C, HW), F16, kind="ExternalInput").ap()
    io["wqt"] = dt("wqt", (C, C), F32R, kind="ExternalInput").ap()
    io["wkt"] = dt("wkt", (C, C), F32R, kind="ExternalInput").ap()
    io["wv"] = dt("wv", (C, C), F16, kind="ExternalInput").ap()
    io["bqr"] = dt("bqr", (1, C), F32R, kind="ExternalInput").ap()
    io["bkr"] = dt("bkr", (1, C), F32R, kind="ExternalInput").ap()
    io["nbkr"] = dt("nbkr", (1, C), F32, kind="ExternalInput").ap()
    io["bv16"] = dt("bv16", (P, C), F16, kind="ExternalInput").ap()
    io["id16"] = dt("id16", (P, P), F16, kind="ExternalInput").ap()
    io["id32"] = dt("id32", (P, P), F32R, kind="ExternalInput").ap()
    io["ones16"] = dt("ones16", (P, 1), F16, kind="ExternalInput").ap()
    io["out"] = dt("out", (C, HW), F16, kind="ExternalOutput").ap()
    with tile.TileContext(nc) as tc:
        for _ in range(repeat):
            _body(tc, nc, io)
    nc.compile()
    return nc


_NC_CACHE = None


def get_nc():
    global _NC_CACHE
    if _NC_CACHE is None:
        _NC_CACHE = _build_nc()
    return _NC_CACHE


def prep_in_maps(x, wq, bq, wk, bk, wv, bv):
    """Host-side input prep: reshape/transpose/dtype casts only."""
    x = np.asarray(x, dtype=np.float32)
    X = x.reshape(B, C, HW)
    xt = X.transpose(0, 2, 1).astype(np.float16)
    xt16 = np.concatenate(
        [xt, np.ones((B, HW, 1), dtype=np.float16)], axis=2
    )
    x16 = X.astype(np.float16)
    wqt = np.ascontiguousarray(np.asarray(wq, np.float32).T)
    wkt = np.ascontiguousarray(np.asarray(wk, np.float32).T)
    wv16 = np.asarray(wv, np.float32).astype(np.float16)
    bqr = np.asarray(bq, np.float32).reshape(1, C)
    bkr = np.asarray(bk, np.float32).reshape(1, C)
    nbkr = (float(HW) * np.asarray(bk, np.float32)).reshape(1, C)
    bv16 = np.tile(np.asarray(bv, np.float32).reshape(1, C), (P, 1)).astype(np.float16)
    id16 = np.eye(P, dtype=np.float16)
    id32 = np.eye(P, dtype=np.float32)
    ones16 = np.ones((P, 1), dtype=np.float16)
    in_maps = []
    for b in range(B):
        in_maps.append(
            {
                "xt16": xt16[b],
                "x16": np.ascontiguousarray(x16[b]),
                "wqt": wqt,
                "wkt": wkt,
                "wv": wv16,
                "bqr": bqr,
                "bkr": bkr,
                "nbkr": nbkr,
                "bv16": bv16,
                "id16": id16,
                "id32": id32,
                "ones16": ones16,
            }
        )
    return in_maps


def kernel(x, wq, bq, wk, bk, wv, bv):
    nc = get_nc()
    in_maps = prep_in_maps(x, wq, bq, wk, bk, wv, bv)
    res = run_bass_kernel_spmd(nc, in_maps, core_ids=list(range(B)))
    out = np.stack([res.results[b]["out"] for b in range(B)])
    return out.reshape(B, C, 64, 64).astype(np.float32)


# revision 86
# speedup vs baseline: 1.0019x; 1.0019x over previous
"""Channel-attention (nn_ChannelAttentionModule) Trainium2 kernel.

Math (per batch b, C=512, N=64*64=4096):
    X = x[b]  [C, N]
    q = Wq X + bq ; k = Wk X + bk ; v = Wv X + bv
    L = q k^T ; A = softmax(L, -1) ; out = A v + X

Restructure:
    L  = Wq G Wk^T + bq (Wk S + N bk)^T + (Wq S) bk^T
         with G = X X^T (fp16 Gram, symmetric; upper block-triangle on the
         PE, lower blocks mirrored via PE transposes) and S = X 1 (spatial
         row sums, accumulated as free ap=1 PE matmuls against ones).
         T1 = G Wk^T, L = Wq T1, and the rank-1 bias terms all run as
         single fp32r passes (1 cyc/row at ap>=256, full fp32 inputs).
    A v = rcp (.) [ (E Wv) X + (E bv) 1^T ]
         with E = exp(L - rowmax) fp16, rcp = 1/rowsum(E).  Reassociating
         (E Wv) X kills the C^2 N v-conv entirely: M0 = E Wv is only C^3,
         computed strip-wise from PE-transposed E^T; E bv is a DVE row-dot.
    out = Identity(o_ps * rcp + rcp*E bv) + X   (scalar-engine scale/bias,
         vector-engine residual add, fp16 store; host casts to fp32).

Sharding: pure data-parallel, one batch per NeuronCore (B=8, 8 cores).
"""

import numpy as np

import concourse.mybir as mybir
import concourse.tile as tile
from concourse import bacc
from concourse.bass_utils import run_bass_kernel_spmd

F32 = mybir.dt.float32
F32R = mybir.dt.float32r
F16 = mybir.dt.float16
AX = mybir.AxisListType.X
EXP = mybir.ActivationFunctionType.Exp
IDENT = mybir.ActivationFunctionType.Identity

B = 8
C = 512
HW = 64 * 64
P = 128
CH = C // P  # 4 channel chunks
NG = 8  # xt granules (4 spatial tiles of 128 rows each)
# upper-triangle start per G row chunk
USTART = [0, 128, 256, 384]
# lower blocks (c,d) mirrored from upper gh[d][:, c-block]
MIRROR = [(1, 0), (2, 0), (2, 1), (3, 0), (3, 1), (3, 2)]
OTAGS = ["pb0", "pb1", "pb2", "pb3", "pa0", "pa1", "pa2", "pa3"]


def _body(tc, nc, io):
    xt16, x16 = io["xt16"], io["x16"]
    wqt, wkt, wv = io["wqt"], io["wkt"], io["wv"]
    bqr, bkr, nbkr, bv16 = io["bqr"], io["bkr"], io["nbkr"], io["bv16"]
    id16, id32, ones16, out = io["id16"], io["id32"], io["ones16"], io["out"]

    ps = tc.alloc_tile_pool(name="ps", bufs=1, space="PSUM")
    sb = tc.alloc_tile_pool(name="sb", bufs=1)

    # ---- persistent SBUF tiles ----
    x16_sb = [sb.tile([P, HW], F16, name=f"x16_{i}", tag=f"x16_{i}") for i in range(CH)]
    wqt_sb = sb.tile([P, CH * C], F32R, name="wqt_sb", tag="wqt_sb")
    wkt_sb = sb.tile([P, CH * C], F32R, name="wkt_sb", tag="wkt_sb")
    wv_sb = sb.tile([P, CH * C], F16, name="wv_sb", tag="wv_sb")
    bvb_sb = sb.tile([P, C], F16, name="bvb_sb", tag="bvb_sb")
    id16_sb = sb.tile([P, P], F16, name="id16_sb", tag="id16_sb")
    id32_sb = sb.tile([P, P], F32R, name="id32_sb", tag="id32_sb")
    ones_sb = sb.tile([P, 1], F16, name="ones_sb", tag="ones_sb")
    CA = C + 1  # x columns + a ones column (S rides along in the G matmuls)
    ar_sb = [sb.tile([P, 4 * CA], F16, name=f"ar{g}", tag=f"ar{g}") for g in range(NG)]
    gf = [sb.tile([P, C], F32R, name=f"gf{i}", tag=f"gf{i}") for i in range(CH)]
    t1f = [sb.tile([P, C], F32R, name=f"t1f{i}", tag=f"t1f{i}") for i in range(CH)]
    e16 = [sb.tile([P, C], F16, name=f"e16_{i}", tag=f"e16_{i}") for i in range(CH)]
    et_sb = [sb.tile([P, C], F16, name=f"et{j}", tag=f"et{j}") for j in range(CH)]
    m0t = [sb.tile([P, C], F16, name=f"m0t{j}", tag=f"m0t{j}") for j in range(CH)]
    s32 = [sb.tile([P, 1], F32R, name=f"s32_{i}", tag=f"s32_{i}") for i in range(CH)]
    bq_sb = sb.tile([1, C], F32R, name="bq_sb", tag="bq_sb")
    bk_sb = sb.tile([1, C], F32R, name="bk_sb", tag="bk_sb")
    u2n_sb = sb.tile([1, C], F32R, name="u2n_sb", tag="u2n_sb")
    nbkr_sb = sb.tile([1, C], F32, name="nbkr_sb", tag="nbkr_sb")
    u1f_sb = sb.tile([1, C], F32R, name="u1f_sb", tag="u1f_sb")

    def wslice(tile_, e, lo, hi):
        return tile_[:, e * C + lo : e * C + hi]

    # ---- DMA schedule (SP queue order == DMA device order) ----
    xtr3 = xt16.rearrange("(g t p) c -> g p t c", p=P, t=4)
    for q in range(4):  # quarter granule 0 so G starts ASAP
        nc.sync.dma_start(ar_sb[0][:, q * CA : (q + 1) * CA], xtr3[0][:, q])
    nc.sync.dma_start(
        ar_sb[1][:, 0 : 2 * CA].rearrange("p (t c) -> p t c", t=2), xtr3[1][:, 0:2]
    )
    nc.sync.dma_start(
        ar_sb[1][:, 2 * CA :].rearrange("p (t c) -> p t c", t=2), xtr3[1][:, 2:4]
    )
    nc.sync.dma_start(
        ar_sb[2][:, 0 : 2 * CA].rearrange("p (t c) -> p t c", t=2), xtr3[2][:, 0:2]
    )
    nc.sync.dma_start(
        ar_sb[2][:, 2 * CA :].rearrange("p (t c) -> p t c", t=2), xtr3[2][:, 2:4]
    )
    nc.sync.dma_start(
        ar_sb[3][:, 0 : 2 * CA].rearrange("p (t c) -> p t c", t=2), xtr3[3][:, 0:2]
    )
    nc.sync.dma_start(
        ar_sb[3][:, 2 * CA :].rearrange("p (t c) -> p t c", t=2), xtr3[3][:, 2:4]
    )
    nc.sync.dma_start(
        ar_sb[4][:, 0 : 2 * CA].rearrange("p (t c) -> p t c", t=2), xtr3[4][:, 0:2]
    )
    nc.sync.dma_start(
        ar_sb[4][:, 2 * CA :].rearrange("p (t c) -> p t c", t=2), xtr3[4][:, 2:4]
    )
    for g in range(5, NG):
        nc.sync.dma_start(ar_sb[g].rearrange("p (t c) -> p t c", t=4), xtr3[g])
    nc.sync.dma_start(id16_sb, id16)
    nc.sync.dma_start(id32_sb, id32)
    nc.sync.dma_start(ones_sb, ones16)
    nc.sync.dma_start(
        wkt_sb.rearrange("p (e c) -> p e c", e=CH),
        wkt.rearrange("(e p) c -> p e c", p=P),
    )
    nc.sync.dma_start(
        wqt_sb.rearrange("p (e c) -> p e c", e=CH),
        wqt.rearrange("(e p) c -> p e c", p=P),
    )
    nc.sync.dma_start(bq_sb, bqr)
    nc.sync.dma_start(bk_sb, bkr)
    nc.sync.dma_start(nbkr_sb, nbkr)
    nc.sync.dma_start(
        wv_sb.rearrange("p (e c) -> p e c", e=CH),
        wv.rearrange("(e p) c -> p e c", p=P),
    )
    nc.sync.dma_start(bvb_sb, bv16)
    for c in range(CH):
        nc.sync.dma_start(x16_sb[c], x16[c * P : (c + 1) * P, :])

    # ---- G = X X^T (upper block-triangle) + S = X 1, accumulated on PE.
    #      For c>=1 the rhs includes the trailing ones column, so S[c-block]
    #      lands in the psum tile's last column for free; c=0 would need a
    #      513-wide bank, so it keeps a separate ap=1 accumulation. ----
    g_ps = [
        ps.tile([P, C + 1 - USTART[i] if i else C], F32, name=f"gps{i}", tag=f"pa{i}")
        for i in range(CH)
    ]
    s_ps0 = ps.tile([P, 1], F32, name="s_ps0", tag="pb0")

    for g2 in range(NG):
        ar4 = ar_sb[g2]
        for t in range(4):
            n = g2 * 4 + t
            first, last = n == 0, n == 4 * NG - 1
            for c in range(CH):
                u = USTART[c]
                nc.tensor.matmul(
                    g_ps[c],
                    lhsT=ar4[:, t * CA + c * P : t * CA + (c + 1) * P],
                    rhs=ar4[:, t * CA + u : t * CA + (C + 1 if c else C)],
                    start=first,
                    stop=last,
                )
            nc.tensor.matmul(
                s_ps0,
                lhsT=ar4[:, t * CA : t * CA + P],
                rhs=ones_sb,
                start=first,
                stop=last,
            )

    # ---- gf = fp32 G in SBUF; mirror lower blocks via fp32 PE transposes ----
    nc.vector.tensor_copy(s32[0], s_ps0)
    for i in range(1, CH):
        nc.vector.tensor_copy(s32[i], g_ps[i][:, C - USTART[i] : C + 1 - USTART[i]])
    # PSUM->SBUF copies split across Act/DVE; the sub-blocks the mirror
    # transposes read are copied first so the mirrors can start early.
    # (g_ps[c] for c>=1 is USTART-offset and 1 col wider for the S column.)
    def gcopy(c, lo, hi):
        u = USTART[c]
        dst = gf[c][:, lo:hi]
        srcp = g_ps[c][:, lo - u : hi - u] if c else g_ps[c][:, lo:hi]
        if c % 2 == 0:
            nc.scalar.copy(dst, srcp)
        else:
            nc.vector.tensor_copy(dst, srcp)

    gcopy(0, 128, 256)   # feeds mirror (1,0)
    gcopy(1, 256, 384)   # feeds mirror (2,1)
    gcopy(0, 256, 512)   # feeds mirrors (2,0), (3,0)
    gcopy(1, 384, 512)   # feeds mirror (3,1)
    gcopy(2, 384, 512)   # feeds mirror (3,2)
    gcopy(0, 0, 128)
    gcopy(1, 128, 256)
    gcopy(2, 256, 384)
    gcopy(3, 384, 512)
    for idx, (c, d) in enumerate(MIRROR):
        tb = ps.tile(
            [P, P], F32R, name="tb", tag="pb3" if idx % 2 == 0 else "pb0"
        )
        nc.tensor.transpose(tb, gf[d][:, c * P : (c + 1) * P], id32_sb)
        nc.scalar.copy(gf[c][:, d * P : (d + 1) * P], tb)

    # ---- u1 = (Wq S)^T, u2 = (Wk S)^T (fp32r); rank-2 bias factors ----
    u1_ps = ps.tile([1, C], F32, name="u1_ps", tag="pb1")
    u2_ps = ps.tile([1, C], F32, name="u2_ps", tag="pb2")
    for e in range(CH):
        nc.tensor.matmul(
            u1_ps, lhsT=s32[e], rhs=wslice(wqt_sb, e, 0, C),
            start=e == 0, stop=e == CH - 1,
        )
    for e in range(CH):
        nc.tensor.matmul(
            u2_ps, lhsT=s32[e], rhs=wslice(wkt_sb, e, 0, C),
            start=e == 0, stop=e == CH - 1,
        )
    nc.scalar.copy(u1f_sb, u1_ps)
    nc.vector.tensor_add(u2n_sb, u2_ps, nbkr_sb)

    # ---- T1 = G Wk^T (single fp32r pass; lhsT = G blocks via symmetry),
    #      interleaved with the logits groups so exp starts ASAP ----
    t1_ps = [ps.tile([P, C], F32, name=f"t1ps{e}", tag=f"pa{e}") for e in range(CH)]

    # ---- logits = Wq T1 + rank-2 (all fp32r); softmax row stats ----
    l_ps = [ps.tile([P, C], F32, name=f"lps{c}", tag=f"pb{c}") for c in range(CH)]
    negmx = [sb.tile([P, 1], F32, name=f"negmx{c}", tag=f"negmx{c}") for c in range(CH)]
    ssum = [sb.tile([P, 1], F32, name=f"ssum{c}", tag=f"ssum{c}") for c in range(CH)]
    rcp = [sb.tile([P, 1], F32, name=f"rcp{c}", tag=f"rcp{c}") for c in range(CH)]
    r0f = [sb.tile([P, 1], F32, name=f"r0f{c}", tag=f"r0f{c}") for c in range(CH)]
    rr = [sb.tile([P, 1], F32, name=f"rr{c}", tag=f"rr{c}") for c in range(CH)]
    for e in range(CH):
        for f in range(CH):
            nc.tensor.matmul(
                t1_ps[e], lhsT=gf[f][:, e * P : (e + 1) * P],
                rhs=wslice(wkt_sb, f, 0, C), start=f == 0, stop=f == CH - 1,
            )
        if e % 2 == 1:
            nc.scalar.copy(t1f[e], t1_ps[e])
        else:
            nc.vector.tensor_copy(t1f[e], t1_ps[e])
    for c in range(CH):
        for e in range(CH):
            nc.tensor.matmul(
                l_ps[c], lhsT=wslice(wqt_sb, e, c * P, (c + 1) * P),
                rhs=t1f[e], start=e == 0, stop=False,
            )
        # rank-1 bias terms last: bq (x) (u2 + N bk)  and  u1 (x) bk
        nc.tensor.matmul(
            l_ps[c], lhsT=bq_sb[:, c * P : (c + 1) * P], rhs=u2n_sb,
            start=False, stop=False,
        )
        nc.tensor.matmul(
            l_ps[c], lhsT=u1f_sb[:, c * P : (c + 1) * P], rhs=bk_sb,
            start=False, stop=True,
        )
        nc.vector.reduce_max(negmx[c], l_ps[c], axis=AX, negate=True)
        nc.scalar.activation(
            e16[c], l_ps[c], EXP, bias=negmx[c], scale=1.0, accum_out=ssum[c]
        )
        nc.vector.reciprocal(rcp[c], ssum[c])

    # ---- E^T via fp16 PE transposes + M0^T = Wv^T E^T, strip-pipelined ----
    et_ps = [ps.tile([P, C], F16, name=f"etps{j}", tag=f"pb{j}") for j in range(CH)]
    m0t_ps = [ps.tile([P, C], F32, name=f"m0tps{d}", tag=f"pa{d}") for d in range(CH)]
    def strip_transp(c):
        cs = slice(c * P, (c + 1) * P)
        for j in range(CH):
            nc.tensor.transpose(
                et_ps[j][:, cs], e16[c][:, j * P : (j + 1) * P], id16_sb
            )
        for j in range(CH):
            if j % 2 == 0:
                nc.scalar.copy(et_sb[j][:, cs], et_ps[j][:, cs])
            else:
                nc.vector.tensor_copy(et_sb[j][:, cs], et_ps[j][:, cs])

    def strip_m0t(c):
        cs = slice(c * P, (c + 1) * P)
        for d in range(CH):
            for e in range(CH):
                nc.tensor.matmul(
                    m0t_ps[d][:, cs], lhsT=wslice(wv_sb, e, d * P, (d + 1) * P),
                    rhs=et_sb[e][:, cs], start=e == 0, stop=e == CH - 1,
                )
        for d in range(CH):
            if d % 2 == 0:
                nc.scalar.copy(m0t[d][:, cs], m0t_ps[d][:, cs])
            else:
                nc.vector.tensor_copy(m0t[d][:, cs], m0t_ps[d][:, cs])

    strip_transp(0)
    strip_transp(1)
    strip_m0t(0)
    strip_transp(2)
    strip_m0t(1)
    strip_transp(3)
    strip_m0t(2)
    strip_m0t(3)

    # ---- rr = rcp * (E bv) via DVE row-dot ----
    for c in range(CH):
        ttrs = sb.tile([P, C], F16, name="ttrs", tag="ttrs", bufs=2)
        nc.vector.tensor_tensor(ttrs, e16[c], bvb_sb, mybir.AluOpType.mult)
        nc.vector.reduce_sum(r0f[c], ttrs, axis=AX)
        nc.vector.tensor_scalar_mul(rr[c], r0f[c], rcp[c])

    # ---- out = rcp*(M0^T^T X) + rr + X, fp16 store per 2-tile chunk ----
    for c in range(CH):
        o16 = sb.tile([P, HW], F16, name="o16", tag="o16", bufs=2)
        for nt in range(HW // 512):
            o_ps = ps.tile(
                [P, 512], F32, name="o_ps", tag=OTAGS[(c * 8 + nt) % 8]
            )
            for d in range(CH):
                nc.tensor.matmul(
                    o_ps,
                    lhsT=m0t[d][:, c * P : (c + 1) * P],
                    rhs=x16_sb[d][:, nt * 512 : (nt + 1) * 512],
                    start=d == 0,
                    stop=d == CH - 1,
                )
            t_sb = sb.tile([P, 512], F16, name="t_sb", tag="t_sb", bufs=4)
            nc.scalar.activation(t_sb, o_ps, IDENT, bias=rr[c], scale=rcp[c])
            nc.vector.tensor_add(
                o16[:, nt * 512 : (nt + 1) * 512],
                t_sb,
                x16_sb[c][:, nt * 512 : (nt + 1) * 512],
            )
            if c == CH - 1:  # tail: store per tile so the last chunk is small
                q = nt * 512
                eng = nc.gpsimd if nt % 2 == 0 else nc.sync
                eng.dma_start(
                    out[c * P : (c + 1) * P, q : q + 512], o16[:, q : q + 512]
                )
            elif nt % 2 == 1:
                q = (nt - 1) * 512
                eng = nc.gpsimd if (c * 4 + nt // 2) % 2 == 0 else nc.sync
                eng.dma_start(
                    out[c * P : (c + 1) * P, q : q + 1024], o16[:, q : q + 1024]
                )

    for pool in (sb, ps):
        pool.release()


def _build_nc(repeat=1):
    nc = bacc.Bacc(
        "TRN2",
        target_bir_lowering=False,
        debug=False,
        num_devices=B,
        enable_asserts=False,
    )
    io = {}
    dt = nc.dram_tensor
    io["xt16"] = dt("xt16", (HW, C + 1), F16, kind="ExternalInput").ap()
    io["x16"] = dt("x16", (C, HW), F16, kind="ExternalInput").ap()
    io["wqt"] = dt("wqt", (C, C), F32R, kind="ExternalInput").ap()
    io["wkt"] = dt("wkt", (C, C), F32R, kind="ExternalInput").ap()
    io["wv"] = dt("wv", (C, C), F16, kind="ExternalInput").ap()
    io["bqr"] = dt("bqr", (1, C), F32R, kind="ExternalInput").ap()
    io["bkr"] = dt("bkr", (1, C), F32R, kind="ExternalInput").ap()
    io["nbkr"] = dt("nbkr", (1, C), F32, kind="ExternalInput").ap()
    io["bv16"] = dt("bv16", (P, C), F16, kind="ExternalInput").ap()
    io["id16"] = dt("id16", (P, P), F16, kind="ExternalInput").ap()
    io["id32"] = dt("id32", (P, P), F32R, kind="ExternalInput").ap()
    io["ones16"] = dt("ones16", (P, 1), F16, kind="ExternalInput").ap()
    io["out"] = dt("out", (C, HW), F16, kind="ExternalOutput").ap()
    with tile.TileContext(nc) as tc:
        for _ in range(repeat):
            _body(tc, nc, io)
    nc.compile()
    return nc


_NC_CACHE = None


def get_nc():
    global _NC_CACHE
    if _NC_CACHE is None:
        _NC_CACHE = _build_nc()
    return _NC_CACHE


def prep_in_maps(x, wq, bq, wk, bk, wv, bv):
    """Host-side input prep: reshape/transpose/dtype casts only."""
    x = np.asarray(x, dtype=np.float32)
    X = x.reshape(B, C, HW)
    xt = X.transpose(0, 2, 1).astype(np.float16)
    xt16 = np.concatenate(
        [xt, np.ones((B, HW, 1), dtype=np.float16)], axis=2
    )
    x16 = X.astype(np.float16)
    wqt = np.ascontiguousarray(np.asarray(wq, np.float32).T)
    wkt = np.ascontiguousarray(np.asarray(wk, np.float32).T)
    wv16 = np.asarray(wv, np.float32).astype(np.float16)
    bqr = np.asarray(bq, np.float32).reshape(1, C)
    bkr = np.asarray(bk, np.float32).reshape(1, C)
    nbkr = (float(HW) * np.asarray(bk, np.float32)).reshape(1, C)
    bv16 = np.tile(np.asarray(bv, np.float32).reshape(1, C), (P, 1)).astype(np.float16)
    id16 = np.eye(P, dtype=np.float16)
    id32 = np.eye(P, dtype=np.float32)
    ones16 = np.ones((P, 1), dtype=np.float16)
    in_maps = []
    for b in range(B):
        in_maps.append(
            {
                "xt16": xt16[b],
                "x16": np.ascontiguousarray(x16[b]),
                "wqt": wqt,
                "wkt": wkt,
                "wv": wv16,
                "bqr": bqr,
                "bkr": bkr,
                "nbkr": nbkr,
                "bv16": bv16,
                "id16": id16,
                "id32": id32,
                "ones16": ones16,
            }
        )
    return in_maps


def kernel(x, wq, bq, wk, bk, wv, bv):
    nc = get_nc()
    in_maps = prep_in_maps(x, wq, bq, wk, bk, wv, bv)
    res = run_bass_kernel_spmd(nc, in_maps, core_ids=list(range(B)))
    out = np.stack([res.results[b]["out"] for b in range(B)])
    return out.reshape(B, C, 64, 64).astype(np.float32)
